# revision 8
# baseline (speedup 1.0000x reference)
"""Trainium2 Bass kernel for the ConLoss module (8-core SPMD).

kernel(**inputs) takes FULL unsharded numpy inputs
(output [64,64,32,100] f32, confidence [30000,32,100] f32,
batch_index [64] int, x_mask [64,32,100] bool, topk scalar) and returns
the FULL outputs (loss, atten, logit_m, pseudo, conf, new_confidence).

Sharding: data-parallel over b1.  Each of the 8 cores gets 8 b1 rows =
256 (b1,q) softmax rows of length b2*k = 6400.  The host pre-transposes
output to (b1, q, b2, k) so all big DMAs are contiguous.  The
confidence buffer itself never moves: only the 64 gathered rows
confidence[batch_index] are shipped (819 KB); the device computes the
EMA-updated rows and the host scatters them into a copy of confidence.
"""

import numpy as np

import concourse.bass as bass
from concourse import mybir
from concourse.bass_utils import run_bass_kernel_spmd

F32 = mybir.dt.float32
AX = mybir.AxisListType
OP = mybir.AluOpType
AF = mybir.ActivationFunctionType

TEMPERATURE = 0.07
BASE_TEMPERATURE = 1.0
CONF_EMA_M = 0.99

B, N, Q, K = 64, 30000, 32, 100
NCORES = 8
BLOC = B // NCORES          # b1 rows per core
R = BLOC * Q                # 256 (b1,q) rows per core
NBK = B * K                 # 6400 softmax length
P = 128                     # partitions per big tile
NT = R // P                 # 2 big tiles per core

CIN = 4 * K                 # sm_in cols:  xd | cr | xm | pen
COUT = 4 * K + 2            # sm_out cols: lm | po | co | up | red(2)

NEG_FMAX = -float(np.finfo(np.float32).max)

# per-rep semaphore totals
NV = 44                     # DVE ops per rep
NA = 6                      # ACT ops per rep
NST = 4 * 16                # store sem per rep


class _Stream:
    """Per-engine stream helper.

    Cross-engine waits are emitted as standalone SemWait instructions
    before the op (the ISA allows only one attached wait per
    instruction); the same-engine serialization wait is attached to the
    op itself; the op increments the stream's own semaphore on
    completion.
    """

    def __init__(self, eng, sem, inc=1, serialize=False):
        self.eng = eng
        self.sem = sem
        self.inc = inc
        self.n = 0
        self.serialize = serialize

    def op(self, waits, fn):
        for s, v in waits:
            if v > 0:
                self.eng.wait_ge(s, v)
        inst = fn()
        if self.serialize and self.n > 0:
            inst._wait_ge(self.sem, self.n)
        inst.then_inc(self.sem, self.inc)
        self.n += self.inc
        return inst


def _build(topk: int, reps: int = 1):
    inv_t = 1.0 / TEMPERATURE
    nc = bass.Bass(trn_type="TRN2")

    xt = nc.dram_tensor("xt", [R, NBK], F32, kind="ExternalInput")
    sin = nc.dram_tensor("sin", [R, CIN], F32, kind="ExternalInput")
    at = nc.dram_tensor("at", [R, NBK], F32, kind="ExternalOutput")
    sout = nc.dram_tensor("sout", [R, COUT], F32, kind="ExternalOutput")

    import contextlib

    ctx = contextlib.ExitStack()

    def sb(name, shape):
        return ctx.enter_context(nc.sbuf_tensor(name, shape, F32))

    X = [sb(f"X{h}", [P, NBK]) for h in range(NT)]
    A = [sb(f"A{h}", [P, NBK]) for h in range(NT)]
    SIN = [sb(f"SIN{h}", [P, CIN]) for h in range(NT)]
    SM = [sb(f"SM{h}", [P, COUT]) for h in range(NT)]
    PT = [sb(f"PT{h}", [P, K]) for h in range(NT)]     # pseudo pre-mask / scratch
    TM = [sb(f"TM{h}", [P, K]) for h in range(NT)]     # scratch (topk mask, hard)
    MK = [sb(f"MK{h}", [P, K]) for h in range(NT)]     # final target mask
    CRS = [sb(f"CRS{h}", [P, K]) for h in range(NT)]   # 0.99*cr
    LG = [sb(f"LG{h}", [P, K]) for h in range(NT)]     # logit (diag log-softmax)
    E2 = [sb(f"E2{h}", [P, K]) for h in range(NT)]     # exp(logit_m - max2)
    SCR = sb("SCR", [P, K])                            # stt main-out scratch
    T8 = [sb(f"T8{h}", [P, 8]) for h in range(NT)]
    M = [sb(f"M{h}", [P, 1]) for h in range(NT)]
    NM = [sb(f"NM{h}", [P, 1]) for h in range(NT)]
    S = [sb(f"S{h}", [P, 1]) for h in range(NT)]
    L = [sb(f"L{h}", [P, 1]) for h in range(NT)]
    NORM = [sb(f"NORM{h}", [P, 1]) for h in range(NT)]
    M2 = [sb(f"M2{h}", [P, 1]) for h in range(NT)]
    NM2 = [sb(f"NM2{h}", [P, 1]) for h in range(NT)]
    S2 = [sb(f"S2{h}", [P, 1]) for h in range(NT)]
    REC = [sb(f"REC{h}", [P, 1]) for h in range(NT)]

    def xd(h):
        return SIN[h][:, 0:K]

    def cr(h):
        return SIN[h][:, K:2 * K]

    def xm(h):
        return SIN[h][:, 2 * K:3 * K]

    def pen(h):
        return SIN[h][:, 3 * K:4 * K]

    def o_lm(h):
        return SM[h][:, 0:K]

    def o_po(h):
        return SM[h][:, K:2 * K]

    def o_co(h):
        return SM[h][:, 2 * K:3 * K]

    def o_up(h):
        return SM[h][:, 3 * K:4 * K]

    def o_red0(h):
        return SM[h][:, 4 * K:4 * K + 1]

    def o_red1(h):
        return SM[h][:, 4 * K + 1:4 * K + 2]

    with (
        nc.Block() as block,
        nc.semaphore("sI") as sI,       # sm_in loads (2 x +16 per rep)
        nc.semaphore("sX0") as sX0,     # X0 load (+16 per rep)
        nc.semaphore("sX1") as sX1,     # X1 load (+16 per rep)
        nc.semaphore("sS") as sS,       # stores (4 x +16 per rep)
        nc.semaphore("sV") as sV,       # DVE ops
        nc.semaphore("sA") as sA,       # ACT ops
    ):

        @block.sync
        def _(eng: bass.BassEngine):
            for r in range(reps):
                vv, av = r * NV, r * NA
                if r:
                    # WAR: prior rep readers of SIN/X done before reload
                    eng.wait_ge(sV, vv)
                    eng.wait_ge(sA, av)
                eng.dma_start(out=SIN[0][:], in_=sin[0:P, :]).then_inc(sI, 16)
                eng.dma_start(out=SIN[1][:], in_=sin[P:R, :]).then_inc(sI, 16)
                eng.dma_start(out=X[0][:], in_=xt[0:P, :]).then_inc(sX0, 16)
                eng.dma_start(out=X[1][:], in_=xt[P:R, :]).then_inc(sX1, 16)
            eng.wait_ge(sI, reps * 32)
            eng.wait_ge(sX0, reps * 16)
            eng.wait_ge(sX1, reps * 16)

        @block.scalar
        def _(eng: bass.BassEngine):
            st = _Stream(eng, sA, serialize=True)
            for r in range(reps):
                vv = r * NV
                w0 = [(sS, r * NST)] if r else []
                # exp big tile 0 (+ softmax denominator via accum)
                st.op(w0 + [(sV, vv + 16), (sX0, (r + 1) * 16)],
                      lambda: eng.activation(out=A[0][:], in_=X[0][:],
                                             func=AF.Exp, scale=inv_t,
                                             bias=NM[0][:], accum_out=S[0][:]))
                st.op([], lambda: eng.activation(out=L[0][:], in_=S[0][:],
                                                 func=AF.Ln))
                # exp big tile 1
                st.op([(sV, vv + 18), (sX1, (r + 1) * 16)],
                      lambda: eng.activation(out=A[1][:], in_=X[1][:],
                                             func=AF.Exp, scale=inv_t,
                                             bias=NM[1][:], accum_out=S[1][:]))
                st.op([], lambda: eng.activation(out=L[1][:], in_=S[1][:],
                                                 func=AF.Ln))
                # exp(logit_m - max2) for conf softmax
                st.op([(sV, vv + 26)],
                      lambda: eng.activation(out=E2[0][:], in_=o_lm(0),
                                             func=AF.Exp, bias=NM2[0][:],
                                             accum_out=S2[0][:]))
                st.op([(sV, vv + 34)],
                      lambda: eng.activation(out=E2[1][:], in_=o_lm(1),
                                             func=AF.Exp, bias=NM2[1][:],
                                             accum_out=S2[1][:]))
                assert st.n == (r + 1) * NA

        @block.vector
        def _(eng: bass.BassEngine):
            st = _Stream(eng, sV, serialize=True)
            for r in range(reps):
                av = r * NA
                w0 = ([(sS, r * NST)] if r else []) + [(sI, (r + 1) * 32)]
                # ---- small front (1..14): needs only sm_in
                st.op(w0, lambda: eng.tensor_mul(PT[0][:], cr(0), xm(0)))
                st.op([], lambda: eng.tensor_mul(PT[1][:], cr(1), xm(1)))
                st.op([], lambda: eng.max(out=T8[0][:], in_=PT[0][:]))
                st.op([], lambda: eng.max(out=T8[1][:], in_=PT[1][:]))
                st.op([], lambda: eng.tensor_scalar(
                    out=TM[0][:], in0=PT[0][:],
                    scalar1=T8[0][:, topk - 1:topk], scalar2=None,
                    op0=OP.is_ge))
                st.op([], lambda: eng.tensor_scalar(
                    out=TM[1][:], in0=PT[1][:],
                    scalar1=T8[1][:, topk - 1:topk], scalar2=None,
                    op0=OP.is_ge))
                st.op([], lambda: eng.tensor_mul(MK[0][:], TM[0][:], xm(0)))
                st.op([], lambda: eng.tensor_mul(MK[1][:], TM[1][:], xm(1)))
                st.op([], lambda: eng.tensor_mul(o_po(0), PT[0][:], MK[0][:]))
                st.op([], lambda: eng.tensor_mul(o_po(1), PT[1][:], MK[1][:]))
                st.op([], lambda: eng.tensor_scalar(
                    out=CRS[0][:], in0=cr(0), scalar1=CONF_EMA_M,
                    scalar2=None, op0=OP.mult))
                st.op([], lambda: eng.tensor_scalar(
                    out=CRS[1][:], in0=cr(1), scalar1=CONF_EMA_M,
                    scalar2=None, op0=OP.mult))
                st.op([], lambda: eng.tensor_copy(o_red1(0), MK[0][:, 0:1]))
                st.op([], lambda: eng.tensor_copy(o_red1(1), MK[1][:, 0:1]))
                # ---- big tile stats (15..18)
                st.op([(sX0, (r + 1) * 16)],
                      lambda: eng.reduce_max(out=M[0][:], in_=X[0][:],
                                             axis=AX.X))
                st.op([], lambda: eng.tensor_scalar(
                    out=NM[0][:], in0=M[0][:], scalar1=-inv_t, scalar2=None,
                    op0=OP.mult))
                st.op([(sX1, (r + 1) * 16)],
                      lambda: eng.reduce_max(out=M[1][:], in_=X[1][:],
                                             axis=AX.X))
                st.op([], lambda: eng.tensor_scalar(
                    out=NM[1][:], in0=M[1][:], scalar1=-inv_t, scalar2=None,
                    op0=OP.mult))
                assert st.n == r * NV + 18
                # ---- finish tile 0 (19..26)
                st.op([(sA, av + 2)],
                      lambda: eng.scalar_tensor_tensor(
                          out=NORM[0][:], in0=M[0][:], scalar=inv_t,
                          in1=L[0][:], op0=OP.mult, op1=OP.add))
                st.op([], lambda: eng.tensor_scalar(
                    out=A[0][:], in0=X[0][:], scalar1=inv_t,
                    scalar2=NORM[0][:], op0=OP.mult, op1=OP.subtract))
                st.op([], lambda: eng.tensor_scalar(
                    out=LG[0][:], in0=xd(0), scalar1=inv_t,
                    scalar2=NORM[0][:], op0=OP.mult, op1=OP.subtract))
                st.op([], lambda: eng.scalar_tensor_tensor(
                    out=SCR[:], in0=o_po(0), scalar=1.0, in1=LG[0][:],
                    op0=OP.mult, op1=OP.mult, accum_out=o_red0(0)))
                st.op([], lambda: eng.tensor_mul(TM[0][:], LG[0][:], xm(0)))
                st.op([], lambda: eng.tensor_add(o_lm(0), TM[0][:], pen(0)))
                st.op([], lambda: eng.reduce_max(out=M2[0][:], in_=o_lm(0),
                                                 axis=AX.X))
                st.op([], lambda: eng.tensor_scalar(
                    out=NM2[0][:], in0=M2[0][:], scalar1=-1.0, scalar2=None,
                    op0=OP.mult))
                assert st.n == r * NV + 26
                # ---- finish tile 1 (27..34)
                st.op([(sA, av + 4)],
                      lambda: eng.scalar_tensor_tensor(
                          out=NORM[1][:], in0=M[1][:], scalar=inv_t,
                          in1=L[1][:], op0=OP.mult, op1=OP.add))
                st.op([], lambda: eng.tensor_scalar(
                    out=A[1][:], in0=X[1][:], scalar1=inv_t,
                    scalar2=NORM[1][:], op0=OP.mult, op1=OP.subtract))
                st.op([], lambda: eng.tensor_scalar(
                    out=LG[1][:], in0=xd(1), scalar1=inv_t,
                    scalar2=NORM[1][:], op0=OP.mult, op1=OP.subtract))
                st.op([], lambda: eng.scalar_tensor_tensor(
                    out=SCR[:], in0=o_po(1), scalar=1.0, in1=LG[1][:],
                    op0=OP.mult, op1=OP.mult, accum_out=o_red0(1)))
                st.op([], lambda: eng.tensor_mul(TM[1][:], LG[1][:], xm(1)))
                st.op([], lambda: eng.tensor_add(o_lm(1), TM[1][:], pen(1)))
                st.op([], lambda: eng.reduce_max(out=M2[1][:], in_=o_lm(1),
                                                 axis=AX.X))
                st.op([], lambda: eng.tensor_scalar(
                    out=NM2[1][:], in0=M2[1][:], scalar1=-1.0, scalar2=None,
                    op0=OP.mult))
                assert st.n == r * NV + 34
                # ---- conf + upd tile 0 (35..39)
                st.op([(sA, av + 5)],
                      lambda: eng.reciprocal(out=REC[0][:], in_=S2[0][:]))
                st.op([], lambda: eng.scalar_tensor_tensor(
                    out=o_co(0), in0=E2[0][:], scalar=REC[0][:], in1=xm(0),
                    op0=OP.mult, op1=OP.mult))
                st.op([], lambda: eng.tensor_scalar(
                    out=TM[0][:], in0=o_lm(0), scalar1=M2[0][:],
                    scalar2=None, op0=OP.is_equal))
                st.op([], lambda: eng.tensor_mul(PT[0][:], TM[0][:], xm(0)))
                st.op([], lambda: eng.scalar_tensor_tensor(
                    out=o_up(0), in0=PT[0][:], scalar=1.0 - CONF_EMA_M,
                    in1=CRS[0][:], op0=OP.mult, op1=OP.add))
                # ---- conf + upd tile 1 (40..44)
                st.op([(sA, av + 6)],
                      lambda: eng.reciprocal(out=REC[1][:], in_=S2[1][:]))
                st.op([], lambda: eng.scalar_tensor_tensor(
                    out=o_co(1), in0=E2[1][:], scalar=REC[1][:], in1=xm(1),
                    op0=OP.mult, op1=OP.mult))
                st.op([], lambda: eng.tensor_scalar(
                    out=TM[1][:], in0=o_lm(1), scalar1=M2[1][:],
                    scalar2=None, op0=OP.is_equal))
                st.op([], lambda: eng.tensor_mul(PT[1][:], TM[1][:], xm(1)))
                st.op([], lambda: eng.scalar_tensor_tensor(
                    out=o_up(1), in0=PT[1][:], scalar=1.0 - CONF_EMA_M,
                    in1=CRS[1][:], op0=OP.mult, op1=OP.add))
                assert st.n == (r + 1) * NV

        @block.gpsimd
        def _(eng: bass.BassEngine):
            for r in range(reps):
                vv = r * NV
                eng.dma_start(out=at[0:P, :], in_=A[0][:]) \
                    ._wait_ge(sV, vv + 20).then_inc(sS, 16)
                eng.dma_start(out=at[P:R, :], in_=A[1][:]) \
                    ._wait_ge(sV, vv + 28).then_inc(sS, 16)
                eng.dma_start(out=sout[0:P, :], in_=SM[0][:]) \
                    ._wait_ge(sV, vv + 39).then_inc(sS, 16)
                eng.dma_start(out=sout[P:R, :], in_=SM[1][:]) \
                    ._wait_ge(sV, vv + 44).then_inc(sS, 16)
            eng.wait_ge(sS, reps * NST)

    ctx.close()
    return nc


_cache = {}


def _get_nc(topk: int, reps: int = 1):
    key = (topk, reps)
    if key not in _cache:
        _cache[key] = _build(topk, reps)
    return _cache[key]


def _prep_inputs(output, confidence, batch_index, x_mask):
    """Host-side shard prep. Returns in_maps for the 8 cores."""
    out_f = np.ascontiguousarray(output, dtype=np.float32)
    # (b1, b2, q, k) -> (b1, q, b2, k) rows of length b2*k
    xt_all = out_f.transpose(0, 2, 1, 3).reshape(B * Q, NBK)
    # diagonal output[g, g] -> (b1, q, k)
    xd_all = out_f[np.arange(B), np.arange(B)].reshape(B * Q, K)
    cr_all = np.ascontiguousarray(
        confidence[np.asarray(batch_index).astype(np.int64)],
        dtype=np.float32).reshape(B * Q, K)
    xm_all = np.ascontiguousarray(x_mask, dtype=np.float32).reshape(B * Q, K)
    pen_all = (1.0 - xm_all) * np.float32(NEG_FMAX)
    sin_all = np.concatenate([xd_all, cr_all, xm_all, pen_all], axis=1)
    in_maps = []
    for cid in range(NCORES):
        lo, hi = cid * R, (cid + 1) * R
        in_maps.append({
            "xt": np.ascontiguousarray(xt_all[lo:hi]),
            "sin": np.ascontiguousarray(sin_all[lo:hi]),
        })
    return in_maps


def _postprocess(results, output, confidence, batch_index, x_mask):
    """Unshard device outputs into the reference's 6-tuple."""
    at_all = np.concatenate([results[c]["at"] for c in range(NCORES)],
                            axis=0)          # (B*Q, NBK)
    so_all = np.concatenate([results[c]["sout"] for c in range(NCORES)],
                            axis=0)          # (B*Q, COUT)

    atten = at_all.reshape(B, Q, B, K).transpose(0, 2, 1, 3)
    atten = np.ascontiguousarray(atten)

    logit_m = np.ascontiguousarray(so_all[:, 0:K]).reshape(B, Q, K)
    pseudo = np.ascontiguousarray(so_all[:, K:2 * K]).reshape(B, Q, K)
    conf = np.ascontiguousarray(so_all[:, 2 * K:3 * K]).reshape(B, Q, K)
    upd = np.ascontiguousarray(so_all[:, 3 * K:4 * K]).reshape(B, Q, K)
    red = so_all[:, 4 * K:4 * K + 2]

    eps = np.float32(np.finfo(np.float32).eps)
    lsum = np.float32(red[:, 0].sum(dtype=np.float32))
    cnt = np.float32(red[:, 1].sum(dtype=np.float32))
    loss = np.float32(-(lsum / (cnt + eps)) * np.float32(BASE_TEMPERATURE))

    new_confidence = np.array(confidence, dtype=np.float32, copy=True)
    new_confidence[np.asarray(batch_index).astype(np.int64)] = upd

    return loss, atten, logit_m, pseudo, conf, new_confidence


def kernel(output, confidence, batch_index, x_mask, topk):
    kk = min(int(topk), K)
    nc = _get_nc(kk)
    in_maps = _prep_inputs(output, confidence, batch_index, x_mask)
    res = run_bass_kernel_spmd(nc, in_maps, core_ids=list(range(NCORES)))
    return _postprocess(res.results, output, confidence, batch_index, x_mask)
